# revision 49
# baseline (speedup 1.0000x reference)
"""Trainium2 Bass kernel for nn_IntraAgg (GNN mean-neighbor aggregation).

reference:
    valid[b,k] = k < neigh_counts[b]
    out = relu( (sum_k valid[b,k] * features[neigh_idx[b,k]]) / neigh_counts[b] )

Strategy (8 NeuronCores, data-parallel over the batch):
  - shard neigh_idx/neigh_counts along B (8192 -> 1024 per core), replicate
    the features table.
  - The gather is done with the GPSIMD `dma_gather` custom instruction
    (one instruction moves thousands of 256B rows; amortizes the ~1us
    SWDGE fixed cost that dominates per-slot indirect DMAs).  dma_gather
    indices are SIGNED int16, so one instruction can address a 65536-row
    span of the 1M-row table (base at the span midpoint, negative indices
    reach backward).  Hence two phases:

    Phase 1: per core, gather the core's ~17k unique needed rows, sorted
      by table index, with one dma_gather per 64K double-window (16
      windows), then write each window's rows back to an HBM staging
      buffer with a plain (affine) HWDGE DMA.  Staging has < 32K rows, so
      it is fully int16-addressable.
    Phase 2: one dma_gather per 128-node block pulls the block's
      neighbor rows from staging in slot order [128, kj, 64]; a strided
      DVE reduce sums over the kj neighbor slots; ACT applies
      relu(x * (1/count)); result is DMAed out.

  - Nodes are count-sorted per core so block b only needs k_sched[b]
    neighbor slots.  Invalid (k >= count) slots point at a zeroed
    staging row, so no per-block zero-fill or bounds check is needed.
  - Descriptor generation on the GPSIMD Q7 pairs (~10.4ns per 256B row,
    4 queues) is the bottleneck; work is greedy-balanced across the 4
    SWDGE queues and quantized in 128-idx chunks.
"""

import numpy as np

N_NODES = 1_000_000
FEAT_DIM = 64
BATCH = 8192
MAX_NEIGH = 32
N_CORES = 8
BLK = 128     # nodes per block (SBUF partition dim)
DWIN = 65536  # rows addressable by one dma_gather (signed int16 window)

_KERNEL_CACHE = {}
_LAST_SCHED = None  # set by prep_core_inputs; consumed by kernel()


def _split_multi_waits(nc):
    """walrus codegen accepts at most one sync-wait per instruction: hoist
    extra waits onto NoOp instructions inserted just before."""
    import bass_rust

    for fn in nc.m.functions:
        for bb in fn.blocks:
            new_list = []
            for inst in bb.instructions:
                si = inst.sync_info
                if si is not None and si.on_wait is not None and len(si.on_wait) > 1:
                    waits = list(si.on_wait)
                    for j, w in enumerate(waits[:-1]):
                        nop = bass_rust.InstNoOp(name=f"{inst.name}-sw{j}")
                        nop.engine = inst.engine
                        nop.sync_info = bass_rust.SyncInfo(on_wait=[w], on_update=[])
                        new_list.append(nop)
                    inst.sync_info = bass_rust.SyncInfo(
                        on_wait=[waits[-1]], on_update=list(si.on_update or [])
                    )
                new_list.append(inst)
            bb.instructions = new_list


def _win_base(w, n_nodes):
    """Signed-int16 gather base for double-window w: rows [DWIN*w,
    DWIN*(w+1)) get indices row - base in [-32768, 32767]."""
    return min(DWIN * w + DWIN // 2, n_nodes - DWIN // 2)


def build_nc(n_nodes=N_NODES, b_loc=BATCH // N_CORES, k=MAX_NEIGH, d=FEAT_DIM,
             legalize=True, k_sched=None, cw_scheds=None):
    """Build the per-core Bass program (SPMD: same program on every core).

    cw_scheds = (cw0_sched, cw1_sched): per-half staging-block schedule per
    double-window (shared across cores; 0 = window empty, no instruction).
    The batch is split into half 0 (blocks 0..nblk/2-1, high-count nodes)
    and half 1 (rest), each with its own staging region + zero block, so
    phase 2 of half 0 only depends on half-0 staging writes and its
    descriptor generation overlaps phase 1 of half 1 (kills the inter-phase
    bubble waiting on staging-write drain)."""
    from concourse import bass, mybir, library_config
    from concourse.tile import TileContext

    assert b_loc % BLK == 0
    nblk = b_loc // BLK
    assert k_sched is not None and cw_scheds is not None
    assert len(k_sched) == nblk and all(1 <= kj <= k for kj in k_sched)

    cw0_sched, cw1_sched = cw_scheds
    nwin = len(cw0_sched)
    c_max = max(max(cw0_sched), max(cw1_sched))
    h0_rows = 128 * sum(cw0_sched)
    h1_base = h0_rows + 128            # after half-0's zero block
    h1_rows = 128 * sum(cw1_sched)
    ns_tot = h1_base + h1_rows + 128   # + half-1's zero block
    assert ns_tot <= 32767, ns_tot
    s1_cols = sum(8 * c for c in cw0_sched) + sum(8 * c for c in cw1_sched)
    s2_cols = sum(8 * kj for kj in k_sched)   # phase-2 idx cols

    nc = bass.Bass(num_swdge_queues=4)
    feat = nc.declare_dram_parameter("feat", [n_nodes, d], mybir.dt.float32,
                                     isOutput=False)
    gidx = nc.declare_dram_parameter("gidx", [128, s1_cols], mybir.dt.int16,
                                     isOutput=False)
    sidx = nc.declare_dram_parameter("sidx", [128, s2_cols], mybir.dt.int16,
                                     isOutput=False)
    # per-block reciprocal columns: recip[p, b] = 1/count of node (b*128+p)
    recip = nc.declare_dram_parameter("recip", [BLK, nblk], mybir.dt.float32,
                                      isOutput=False)
    out = nc.declare_dram_parameter("out", [b_loc, d], mybir.dt.float32,
                                    isOutput=True)

    fp32 = mybir.dt.float32
    with TileContext(nc) as tc:
        with tc.tile_pool(name="const", bufs=1) as constp, \
             tc.tile_pool(name="stgp", bufs=1, space="DRAM") as stgp, \
             tc.tile_pool(name="p1p", bufs=12) as p1p, \
             tc.tile_pool(name="gp", bufs=8) as gp, \
             tc.tile_pool(name="redp", bufs=4) as redp, \
             tc.tile_pool(name="outp", bufs=4) as outp:
            nc.gpsimd.load_library(library_config.mlp)

            # one shared register per distinct idx-count (to_reg per call
            # would exhaust the register file)
            nreg = {}
            for cw in set(c for c in list(cw0_sched) + list(cw1_sched) if c):
                nreg[128 * cw] = nc.gpsimd.to_reg(128 * cw)
            for kj in k_sched:
                for k0 in range(0, kj, 8):
                    km = min(8, kj - k0)
                    if 128 * km not in nreg:
                        nreg[128 * km] = nc.gpsimd.to_reg(128 * km)

            # split the gidx load so the first window's columns land fast
            # (subtile deps let the first gather start before the rest).
            c0 = 8 * next(c for c in cw0_sched if c)
            gt = constp.tile([128, s1_cols], mybir.dt.int16)
            nc.sync.dma_start(out=gt[:, :c0], in_=gidx[:, :c0])
            nc.sync.dma_start(out=gt[:, c0:], in_=gidx[:, c0:])
            st = constp.tile([128, s2_cols], mybir.dt.int16)
            nc.sync.dma_start(out=st[:], in_=sidx[:, :])
            rt = constp.tile([BLK, nblk], fp32)
            nc.sync.dma_start(out=rt[:], in_=recip[:, :])

            staging = stgp.tile([ns_tot, d], fp32)
            # per-half zero blocks for invalid (k >= count) slots
            zt = constp.tile([128, d], fp32)
            nc.vector.memset(zt[:], 0.0)
            for zp in (h0_rows, h1_base + h1_rows):
                nc.sync.dma_start(
                    out=staging[zp:zp + 128, :].rearrange(
                        "(c p) d -> p c d", p=128),
                    in_=zt[:].rearrange("p (c d) -> p c d", d=d),
                )

            # Greedy-balance the 4 SWDGE queues by cumulative descriptor
            # work: each queue is serviced by its own GPSIMD Q7 core pair
            # and the engine blocks while the target pair is still busy.
            qload = [0] * 4

            def pick_queue(work):
                qn = qload.index(min(qload))
                qload[qn] += work
                return qn

            # ---- phase 1: double-window gathers, half 0 then half 1 ----
            col = 0   # gidx col offset
            for half, (cw_sched, off) in enumerate(
                    ((cw0_sched, 0), (cw1_sched, h1_base))):
                for w in range(nwin):
                    cw = cw_sched[w]
                    if cw == 0:
                        continue
                    n_idx = 128 * cw
                    base = _win_base(w, n_nodes)
                    t1 = p1p.tile([128, c_max * d], fp32, tag="t1")
                    t1v = t1[:, :cw * d].rearrange("p (c d) -> p c d", d=d)
                    nc.gpsimd.dma_gather(
                        t1v,
                        feat[base:min(base + DWIN // 2, n_nodes), :],
                        gt[:, col:col + 8 * cw],
                        n_idx,
                        nreg[n_idx],
                        d,
                        # >64 descriptors/SDMA engine don't fit one packet
                        single_packet=(n_idx <= 1024),
                        queue_num=pick_queue(n_idx),
                    )
                    nc.sync.dma_start(
                        out=staging[off:off + n_idx, :].rearrange(
                            "(c p) d -> p c d", p=128),
                        in_=t1v,
                    )
                    off += n_idx
                    col += 8 * cw

            # ---- phase 2: per-block slot gathers from staging + reduce ----
            # blocks 0..nblk/2-1 read only half-0's staging region (subtile
            # deps), so their descriptor gen overlaps half-1's phase 1.
            # Within each half, process blocks smallest-last-block-first so
            # the phase ends on one mid-size block instead of a trickle of
            # tiny reduces each gated by its chunks' SDMA drain.
            col2_base = {}
            acc = 0
            for b in range(nblk):
                col2_base[b] = acc
                acc += 8 * k_sched[b]
            order2 = (list(range(nblk // 2 - 1, -1, -1))
                      + list(range(nblk - 1, nblk // 2 - 1, -1)))
            for b in order2:
                kj = k_sched[b]
                col2 = col2_base[b]
                sl = slice(b * BLK, (b + 1) * BLK)
                if b < nblk // 2:
                    src = staging[0:h1_base, :]
                else:
                    src = staging[h1_base:ns_tot, :]

                # uniform size: Tile pools size a tag's rotating buffers
                # from the first tile seen, so per-block sizes are only
                # safe when emitted largest-first.
                g = gp.tile([BLK, k * d], fp32, tag="g")
                # split the block's slot gather into <=8-slot (1024-idx)
                # chunks spread over the 4 SWDGE queues
                for k0 in range(0, kj, 8):
                    km = min(8, kj - k0)
                    nc.gpsimd.dma_gather(
                        g[:, k0 * d:(k0 + km) * d].rearrange(
                            "p (k d) -> p k d", d=d),
                        src,
                        st[:, col2 + 8 * k0:col2 + 8 * (k0 + km)],
                        128 * km,
                        nreg[128 * km],
                        d,
                        queue_num=pick_queue(128 * km),
                    )
                red = redp.tile([BLK, d], fp32)
                nc.vector.tensor_reduce(
                    out=red[:],
                    in_=g[:, :kj * d].rearrange("p (k d) -> p d k", d=d),
                    axis=mybir.AxisListType.X,
                    op=mybir.AluOpType.add,
                )
                o = outp.tile([BLK, d], fp32)
                nc.scalar.activation(
                    out=o[:],
                    in_=red[:],
                    func=mybir.ActivationFunctionType.Relu,
                    scale=rt[:, b:b + 1],
                )
                nc.sync.dma_start(out=out[sl, :], in_=o[:])

    if legalize:
        _split_multi_waits(nc)
    # raw Bass skips Bacc's codegen pass for extended/pseudo instructions
    # (dma_gather, load_library); without it walrus sees empty .instr bytes
    # and fails with "ISA wrong length".
    mybir.codegen_inst_isa_subclasses(nc)
    return nc


def _wrap16(vals, cols, fill):
    """Pack `vals` into the [128, cols] int16 wrapped layout dma_gather
    expects: position i at [i%16, i//16], replicated across the 8
    16-partition groups."""
    flat = np.full(16 * cols, fill, dtype=np.int16)
    flat[:len(vals)] = vals
    arr = flat.reshape(cols, 16).T  # position i = col*16 + p at [p, col]
    return np.tile(arr, (8, 1))


def prep_core_inputs(features, neigh_idx, neigh_counts, n_cores=N_CORES):
    """Host-side sharding + index-space preprocessing (no feature data is
    touched on the host).  Returns (in_maps, orders, k_sched) and stores the
    shared phase-1 window schedule in _LAST_SCHED."""
    global _LAST_SCHED
    n_nodes = features.shape[0]
    b = neigh_idx.shape[0]
    b_loc = b // n_cores
    k = neigh_idx.shape[1]
    nblk = b_loc // BLK
    nwin = (n_nodes + DWIN - 1) // DWIN

    idx_all = np.asarray(neigh_idx, dtype=np.int64)
    counts = np.asarray(neigh_counts, dtype=np.int64)
    recip = (1.0 / counts.astype(np.float64)).astype(np.float32)
    feat = np.ascontiguousarray(np.asarray(features, dtype=np.float32))

    # per-core count-sort and per-half unique-row analysis.  Half 0 =
    # count-sorted nodes [0, b_loc/2) (blocks 0..nblk/2-1), half 1 = rest.
    hb = b_loc // 2
    cores = []
    k_sched = np.ones(nblk, dtype=np.int64)
    nw_max = [np.zeros(nwin, dtype=np.int64), np.zeros(nwin, dtype=np.int64)]
    for c in range(n_cores):
        sl = slice(c * b_loc, (c + 1) * b_loc)
        cnt_c = counts[sl]
        order = np.argsort(-cnt_c, kind="stable")
        sorted_cnt = cnt_c[order]
        k_sched = np.maximum(
            k_sched, sorted_cnt.reshape(nblk, BLK).max(axis=1))
        idx_c = idx_all[sl][order]              # [b_loc, k] count-sorted
        valid = (np.arange(k)[None, :] < sorted_cnt[:, None])
        uniqs = []
        for h, rsl in enumerate((slice(0, hb), slice(hb, b_loc))):
            uniq = np.unique(idx_c[rsl][valid[rsl]])
            nw_max[h] = np.maximum(
                nw_max[h], np.bincount(uniq // DWIN, minlength=nwin))
            uniqs.append(uniq)
        cores.append((order, sorted_cnt, idx_c, valid, uniqs))

    cw_scheds = tuple(
        tuple(int(-(-n // BLK)) if n > 0 else 0 for n in nw_max[h])
        for h in range(2))
    h0_rows = 128 * sum(cw_scheds[0])
    h1_base = h0_rows + 128
    h1_rows = 128 * sum(cw_scheds[1])
    # per-half zero-block position, in each half's local index space
    zp_loc = (h0_rows, h1_rows)
    k_sched = tuple(int(x) for x in k_sched)
    _LAST_SCHED = cw_scheds

    in_maps, orders = [], []
    for c in range(n_cores):
        order, sorted_cnt, idx_c, valid, uniqs = cores[c]
        # staging position of each unique row (window-major, padded
        # blocks), in each half's local index space
        pos_uniqs = []
        gidx_cols = []
        for h in range(2):
            uniq = uniqs[h]
            pos_uniq = np.empty(len(uniq), dtype=np.int64)
            off = 0
            for w in range(nwin):
                cw = cw_scheds[h][w]
                if cw == 0:
                    continue
                base = _win_base(w, n_nodes)
                lo = np.searchsorted(uniq, w * DWIN)
                hi = np.searchsorted(uniq, (w + 1) * DWIN)
                nwc = hi - lo
                pos_uniq[lo:hi] = off + np.arange(nwc)
                local = (uniq[lo:hi] - base)
                assert local.size == 0 or (
                    local.min() >= -32768 and local.max() <= 32767)
                # ascending order: trailing (pad) entries must be >= 0 so
                # the ucode's trailing-negative trim never eats real
                # indices; pad by repeating the last (largest) index.
                if nwc > 0:
                    assert local[-1] >= 0, (
                        "window upper half empty; signed-window gather "
                        "would mis-trim")
                    fill = np.int16(local[-1])
                else:
                    fill = np.int16(0)
                gidx_cols.append(
                    _wrap16(local.astype(np.int16), 8 * cw, fill))
                off += 128 * cw
            pos_uniqs.append(pos_uniq)
        gidx16 = np.concatenate(gidx_cols, axis=1)

        # phase-2 slot indices: block-major, position i = k*128 + p,
        # values relative to the block's half's staging base
        sidx_cols = []
        for bb in range(nblk):
            h = 0 if bb < nblk // 2 else 1
            uniq, pos_uniq, zp = uniqs[h], pos_uniqs[h], zp_loc[h]
            kj = k_sched[bb]
            blk_idx = idx_c[bb * BLK:(bb + 1) * BLK, :kj]        # [128, kj]
            blk_valid = valid[bb * BLK:(bb + 1) * BLK, :kj]
            pos = np.full((BLK, kj), zp, dtype=np.int64)
            pos[blk_valid] = pos_uniq[
                np.searchsorted(uniq, blk_idx[blk_valid])]
            # position i = k*128 + p  ->  stream k-major
            stream = pos.T.reshape(-1)                           # [kj*128]
            sidx_cols.append(_wrap16(stream.astype(np.int16), 8 * kj,
                                     np.int16(zp)))
        sidx16 = np.concatenate(sidx_cols, axis=1)

        # recip packed per block: recip_p[p, b] = 1/count of sorted node
        # b*128 + p on this core.
        rc = recip[c * b_loc:(c + 1) * b_loc][order]
        recip_p = rc.reshape(nblk, BLK).T                        # [128, nblk]

        orders.append(order)
        in_maps.append({
            "feat": feat,
            "gidx": np.ascontiguousarray(gidx16),
            "sidx": np.ascontiguousarray(sidx16),
            "recip": np.ascontiguousarray(recip_p),
        })
    return in_maps, orders, k_sched


def kernel(features, neigh_idx, neigh_counts):
    from concourse.bass_utils import run_bass_kernel_spmd

    in_maps, orders, k_sched = prep_core_inputs(
        features, neigh_idx, neigh_counts)
    cw_scheds = _LAST_SCHED
    key = ("nc", N_NODES, BATCH // N_CORES, MAX_NEIGH, FEAT_DIM, k_sched)
    if key not in _KERNEL_CACHE:
        _KERNEL_CACHE[key] = build_nc(k_sched=list(k_sched),
                                      cw_scheds=cw_scheds)
    nc = _KERNEL_CACHE[key]

    res = run_bass_kernel_spmd(nc, in_maps, list(range(N_CORES)))
    b_loc = BATCH // N_CORES
    out = np.empty((BATCH, FEAT_DIM), dtype=np.float32)
    for c in range(N_CORES):
        out_c = np.empty((b_loc, FEAT_DIM), dtype=np.float32)
        out_c[orders[c]] = res.results[c]["out"]
        out[c * b_loc:(c + 1) * b_loc] = out_c
    return out


# revision 50
# speedup vs baseline: 1.0891x; 1.0891x over previous
"""Trainium2 Bass kernel for nn_IntraAgg (GNN mean-neighbor aggregation).

reference:
    valid[b,k] = k < neigh_counts[b]
    out = relu( (sum_k valid[b,k] * features[neigh_idx[b,k]]) / neigh_counts[b] )

Strategy (8 NeuronCores, data-parallel over the batch):
  - shard neigh_idx/neigh_counts along B (8192 -> 1024 per core), replicate
    the features table.
  - The gather is done with the GPSIMD `dma_gather` custom instruction
    (one instruction moves thousands of 256B rows; amortizes the ~1us
    SWDGE fixed cost that dominates per-slot indirect DMAs).  dma_gather
    indices are SIGNED int16, so one instruction can address a 65536-row
    span of the 1M-row table (base at the span midpoint, negative indices
    reach backward).  Hence two phases:

    Phase 1: per core, gather the core's ~17k unique needed rows, sorted
      by table index, with one dma_gather per 64K double-window (16
      windows), then write each window's rows back to an HBM staging
      buffer with a plain (affine) HWDGE DMA.  Staging has < 32K rows, so
      it is fully int16-addressable.
    Phase 2: one dma_gather per 128-node block pulls the block's
      neighbor rows from staging in slot order [128, kj, 64]; a strided
      DVE reduce sums over the kj neighbor slots; ACT applies
      relu(x * (1/count)); result is DMAed out.

  - Nodes are count-sorted per core so block b only needs k_sched[b]
    neighbor slots.  Invalid (k >= count) slots point at a zeroed
    staging row, so no per-block zero-fill or bounds check is needed.
  - Descriptor generation on the GPSIMD Q7 pairs (~10.4ns per 256B row,
    4 queues) is the bottleneck; work is greedy-balanced across the 4
    SWDGE queues and quantized in 128-idx chunks.
"""

import numpy as np

N_NODES = 1_000_000
FEAT_DIM = 64
BATCH = 8192
MAX_NEIGH = 32
N_CORES = 8
BLK = 128     # nodes per block (SBUF partition dim)
DWIN = 65536  # rows addressable by one dma_gather (signed int16 window)

_KERNEL_CACHE = {}
_LAST_SCHED = None  # set by prep_core_inputs; consumed by kernel()


def _split_multi_waits(nc):
    """walrus codegen accepts at most one sync-wait per instruction: hoist
    extra waits onto NoOp instructions inserted just before."""
    import bass_rust

    for fn in nc.m.functions:
        for bb in fn.blocks:
            new_list = []
            for inst in bb.instructions:
                si = inst.sync_info
                if si is not None and si.on_wait is not None and len(si.on_wait) > 1:
                    waits = list(si.on_wait)
                    for j, w in enumerate(waits[:-1]):
                        nop = bass_rust.InstNoOp(name=f"{inst.name}-sw{j}")
                        nop.engine = inst.engine
                        nop.sync_info = bass_rust.SyncInfo(on_wait=[w], on_update=[])
                        new_list.append(nop)
                    inst.sync_info = bass_rust.SyncInfo(
                        on_wait=[waits[-1]], on_update=list(si.on_update or [])
                    )
                new_list.append(inst)
            bb.instructions = new_list


def _win_base(w, n_nodes):
    """Signed-int16 gather base for double-window w: rows [DWIN*w,
    DWIN*(w+1)) get indices row - base in [-32768, 32767]."""
    return min(DWIN * w + DWIN // 2, n_nodes - DWIN // 2)


def build_nc(n_nodes=N_NODES, b_loc=BATCH // N_CORES, k=MAX_NEIGH, d=FEAT_DIM,
             legalize=True, k_sched=None, cw_scheds=None):
    """Build the per-core Bass program (SPMD: same program on every core).

    cw_scheds = (cw0_sched, cw1_sched): per-half staging-block schedule per
    double-window (shared across cores; 0 = window empty, no instruction).
    The batch is split into half 0 (blocks 0..nblk/2-1, high-count nodes)
    and half 1 (rest), each with its own staging region + zero block, so
    phase 2 of half 0 only depends on half-0 staging writes and its
    descriptor generation overlaps phase 1 of half 1 (kills the inter-phase
    bubble waiting on staging-write drain)."""
    from concourse import bass, mybir, library_config
    from concourse.tile import TileContext

    assert b_loc % BLK == 0
    nblk = b_loc // BLK
    assert k_sched is not None and cw_scheds is not None
    assert len(k_sched) == nblk and all(1 <= kj <= k for kj in k_sched)

    cw0_sched, cw1_sched = cw_scheds
    nwin = len(cw0_sched)
    c_max = max(max(cw0_sched), max(cw1_sched))
    h0_rows = 128 * sum(cw0_sched)
    h1_base = h0_rows + 128            # after half-0's zero block
    h1_rows = 128 * sum(cw1_sched)
    ns_tot = h1_base + h1_rows + 128   # + half-1's zero block
    assert ns_tot <= 32767, ns_tot
    s1_cols = sum(8 * c for c in cw0_sched) + sum(8 * c for c in cw1_sched)
    s2_cols = sum(8 * kj for kj in k_sched)   # phase-2 idx cols

    nc = bass.Bass(num_swdge_queues=4)
    feat = nc.declare_dram_parameter("feat", [n_nodes, d], mybir.dt.float32,
                                     isOutput=False)
    gidx = nc.declare_dram_parameter("gidx", [128, s1_cols], mybir.dt.int16,
                                     isOutput=False)
    sidx = nc.declare_dram_parameter("sidx", [128, s2_cols], mybir.dt.int16,
                                     isOutput=False)
    # per-block reciprocal columns: recip[p, b] = 1/count of node (b*128+p)
    recip = nc.declare_dram_parameter("recip", [BLK, nblk], mybir.dt.float32,
                                      isOutput=False)
    out = nc.declare_dram_parameter("out", [b_loc, d], mybir.dt.float32,
                                    isOutput=True)

    fp32 = mybir.dt.float32
    with TileContext(nc) as tc:
        with tc.tile_pool(name="const", bufs=1) as constp, \
             tc.tile_pool(name="stgp", bufs=1, space="DRAM") as stgp, \
             tc.tile_pool(name="p1p", bufs=12) as p1p, \
             tc.tile_pool(name="gp", bufs=8) as gp, \
             tc.tile_pool(name="redp", bufs=4) as redp, \
             tc.tile_pool(name="outp", bufs=4) as outp:
            nc.gpsimd.load_library(library_config.mlp)

            # one shared register per distinct idx-count (to_reg per call
            # would exhaust the register file)
            nreg = {}
            for cw in set(c for c in list(cw0_sched) + list(cw1_sched) if c):
                nreg[128 * cw] = nc.gpsimd.to_reg(128 * cw)
            for kj in k_sched:
                for k0 in range(0, kj, 8):
                    km = min(8, kj - k0)
                    if 128 * km not in nreg:
                        nreg[128 * km] = nc.gpsimd.to_reg(128 * km)

            # split the gidx load so the first window's columns land fast
            # (subtile deps let the first gather start before the rest).
            c0 = 8 * next(c for c in cw0_sched if c)
            gt = constp.tile([128, s1_cols], mybir.dt.int16)
            nc.sync.dma_start(out=gt[:, :c0], in_=gidx[:, :c0])
            nc.sync.dma_start(out=gt[:, c0:], in_=gidx[:, c0:])
            st = constp.tile([128, s2_cols], mybir.dt.int16)
            nc.sync.dma_start(out=st[:], in_=sidx[:, :])
            rt = constp.tile([BLK, nblk], fp32)
            nc.sync.dma_start(out=rt[:], in_=recip[:, :])

            staging = stgp.tile([ns_tot, d], fp32)
            # per-half zero blocks for invalid (k >= count) slots
            zt = constp.tile([128, d], fp32)
            nc.vector.memset(zt[:], 0.0)
            for zp in (h0_rows, h1_base + h1_rows):
                nc.sync.dma_start(
                    out=staging[zp:zp + 128, :].rearrange(
                        "(c p) d -> p c d", p=128),
                    in_=zt[:].rearrange("p (c d) -> p c d", d=d),
                )

            # Greedy-balance the 4 SWDGE queues by cumulative descriptor
            # work: each queue is serviced by its own GPSIMD Q7 core pair
            # and the engine blocks while the target pair is still busy.
            qload = [0] * 4

            def pick_queue(work):
                qn = qload.index(min(qload))
                qload[qn] += work
                return qn

            # ---- phase 1: double-window gathers, half 0 then half 1 ----
            col = 0   # gidx col offset
            for half, (cw_sched, off) in enumerate(
                    ((cw0_sched, 0), (cw1_sched, h1_base))):
                for w in range(nwin):
                    cw = cw_sched[w]
                    if cw == 0:
                        continue
                    n_idx = 128 * cw
                    base = _win_base(w, n_nodes)
                    t1 = p1p.tile([128, c_max * d], fp32, tag="t1")
                    t1v = t1[:, :cw * d].rearrange("p (c d) -> p c d", d=d)
                    nc.gpsimd.dma_gather(
                        t1v,
                        feat[base:min(base + DWIN // 2, n_nodes), :],
                        gt[:, col:col + 8 * cw],
                        n_idx,
                        nreg[n_idx],
                        d,
                        # >64 descriptors/SDMA engine don't fit one packet
                        single_packet=(n_idx <= 1024),
                        queue_num=pick_queue(n_idx),
                    )
                    nc.sync.dma_start(
                        out=staging[off:off + n_idx, :].rearrange(
                            "(c p) d -> p c d", p=128),
                        in_=t1v,
                    )
                    off += n_idx
                    col += 8 * cw

            # ---- phase 2: per-block slot gathers from staging + reduce ----
            # blocks 0..nblk/2-1 read only half-0's staging region (subtile
            # deps), so their descriptor gen overlaps half-1's phase 1.
            col2 = 0
            for b in range(nblk):
                kj = k_sched[b]
                sl = slice(b * BLK, (b + 1) * BLK)
                if b < nblk // 2:
                    src = staging[0:h1_base, :]
                else:
                    src = staging[h1_base:ns_tot, :]

                g = gp.tile([BLK, kj * d], fp32, tag="g")
                # split the block's slot gather into <=8-slot (1024-idx)
                # chunks spread over the 4 SWDGE queues
                for k0 in range(0, kj, 8):
                    km = min(8, kj - k0)
                    nc.gpsimd.dma_gather(
                        g[:, k0 * d:(k0 + km) * d].rearrange(
                            "p (k d) -> p k d", d=d),
                        src,
                        st[:, col2 + 8 * k0:col2 + 8 * (k0 + km)],
                        128 * km,
                        nreg[128 * km],
                        d,
                        queue_num=pick_queue(128 * km),
                    )
                col2 += 8 * kj

                red = redp.tile([BLK, d], fp32)
                nc.vector.tensor_reduce(
                    out=red[:],
                    in_=g[:, :kj * d].rearrange("p (k d) -> p d k", d=d),
                    axis=mybir.AxisListType.X,
                    op=mybir.AluOpType.add,
                )
                o = outp.tile([BLK, d], fp32)
                nc.scalar.activation(
                    out=o[:],
                    in_=red[:],
                    func=mybir.ActivationFunctionType.Relu,
                    scale=rt[:, b:b + 1],
                )
                nc.sync.dma_start(out=out[sl, :], in_=o[:])

    if legalize:
        _split_multi_waits(nc)
    # raw Bass skips Bacc's codegen pass for extended/pseudo instructions
    # (dma_gather, load_library); without it walrus sees empty .instr bytes
    # and fails with "ISA wrong length".
    mybir.codegen_inst_isa_subclasses(nc)
    return nc


def _wrap16(vals, cols, fill):
    """Pack `vals` into the [128, cols] int16 wrapped layout dma_gather
    expects: position i at [i%16, i//16], replicated across the 8
    16-partition groups."""
    flat = np.full(16 * cols, fill, dtype=np.int16)
    flat[:len(vals)] = vals
    arr = flat.reshape(cols, 16).T  # position i = col*16 + p at [p, col]
    return np.tile(arr, (8, 1))


def prep_core_inputs(features, neigh_idx, neigh_counts, n_cores=N_CORES):
    """Host-side sharding + index-space preprocessing (no feature data is
    touched on the host).  Returns (in_maps, orders, k_sched) and stores the
    shared phase-1 window schedule in _LAST_SCHED."""
    global _LAST_SCHED
    n_nodes = features.shape[0]
    b = neigh_idx.shape[0]
    b_loc = b // n_cores
    k = neigh_idx.shape[1]
    nblk = b_loc // BLK
    nwin = (n_nodes + DWIN - 1) // DWIN

    idx_all = np.asarray(neigh_idx, dtype=np.int64)
    counts = np.asarray(neigh_counts, dtype=np.int64)
    recip = (1.0 / counts.astype(np.float64)).astype(np.float32)
    feat = np.ascontiguousarray(np.asarray(features, dtype=np.float32))

    # per-core count-sort and per-half unique-row analysis.  Half 0 =
    # count-sorted nodes [0, b_loc/2) (blocks 0..nblk/2-1), half 1 = rest.
    hb = b_loc // 2
    cores = []
    k_sched = np.ones(nblk, dtype=np.int64)
    nw_max = [np.zeros(nwin, dtype=np.int64), np.zeros(nwin, dtype=np.int64)]
    for c in range(n_cores):
        sl = slice(c * b_loc, (c + 1) * b_loc)
        cnt_c = counts[sl]
        order = np.argsort(-cnt_c, kind="stable")
        sorted_cnt = cnt_c[order]
        k_sched = np.maximum(
            k_sched, sorted_cnt.reshape(nblk, BLK).max(axis=1))
        idx_c = idx_all[sl][order]              # [b_loc, k] count-sorted
        valid = (np.arange(k)[None, :] < sorted_cnt[:, None])
        uniqs = []
        for h, rsl in enumerate((slice(0, hb), slice(hb, b_loc))):
            uniq = np.unique(idx_c[rsl][valid[rsl]])
            nw_max[h] = np.maximum(
                nw_max[h], np.bincount(uniq // DWIN, minlength=nwin))
            uniqs.append(uniq)
        cores.append((order, sorted_cnt, idx_c, valid, uniqs))

    cw_scheds = tuple(
        tuple(int(-(-n // BLK)) if n > 0 else 0 for n in nw_max[h])
        for h in range(2))
    h0_rows = 128 * sum(cw_scheds[0])
    h1_base = h0_rows + 128
    h1_rows = 128 * sum(cw_scheds[1])
    # per-half zero-block position, in each half's local index space
    zp_loc = (h0_rows, h1_rows)
    k_sched = tuple(int(x) for x in k_sched)
    _LAST_SCHED = cw_scheds

    in_maps, orders = [], []
    for c in range(n_cores):
        order, sorted_cnt, idx_c, valid, uniqs = cores[c]
        # staging position of each unique row (window-major, padded
        # blocks), in each half's local index space
        pos_uniqs = []
        gidx_cols = []
        for h in range(2):
            uniq = uniqs[h]
            pos_uniq = np.empty(len(uniq), dtype=np.int64)
            off = 0
            for w in range(nwin):
                cw = cw_scheds[h][w]
                if cw == 0:
                    continue
                base = _win_base(w, n_nodes)
                lo = np.searchsorted(uniq, w * DWIN)
                hi = np.searchsorted(uniq, (w + 1) * DWIN)
                nwc = hi - lo
                pos_uniq[lo:hi] = off + np.arange(nwc)
                local = (uniq[lo:hi] - base)
                assert local.size == 0 or (
                    local.min() >= -32768 and local.max() <= 32767)
                # ascending order: trailing (pad) entries must be >= 0 so
                # the ucode's trailing-negative trim never eats real
                # indices; pad by repeating the last (largest) index.
                if nwc > 0:
                    assert local[-1] >= 0, (
                        "window upper half empty; signed-window gather "
                        "would mis-trim")
                    fill = np.int16(local[-1])
                else:
                    fill = np.int16(0)
                gidx_cols.append(
                    _wrap16(local.astype(np.int16), 8 * cw, fill))
                off += 128 * cw
            pos_uniqs.append(pos_uniq)
        gidx16 = np.concatenate(gidx_cols, axis=1)

        # phase-2 slot indices: block-major, position i = k*128 + p,
        # values relative to the block's half's staging base
        sidx_cols = []
        for bb in range(nblk):
            h = 0 if bb < nblk // 2 else 1
            uniq, pos_uniq, zp = uniqs[h], pos_uniqs[h], zp_loc[h]
            kj = k_sched[bb]
            blk_idx = idx_c[bb * BLK:(bb + 1) * BLK, :kj]        # [128, kj]
            blk_valid = valid[bb * BLK:(bb + 1) * BLK, :kj]
            pos = np.full((BLK, kj), zp, dtype=np.int64)
            pos[blk_valid] = pos_uniq[
                np.searchsorted(uniq, blk_idx[blk_valid])]
            # position i = k*128 + p  ->  stream k-major
            stream = pos.T.reshape(-1)                           # [kj*128]
            sidx_cols.append(_wrap16(stream.astype(np.int16), 8 * kj,
                                     np.int16(zp)))
        sidx16 = np.concatenate(sidx_cols, axis=1)

        # recip packed per block: recip_p[p, b] = 1/count of sorted node
        # b*128 + p on this core.
        rc = recip[c * b_loc:(c + 1) * b_loc][order]
        recip_p = rc.reshape(nblk, BLK).T                        # [128, nblk]

        orders.append(order)
        in_maps.append({
            "feat": feat,
            "gidx": np.ascontiguousarray(gidx16),
            "sidx": np.ascontiguousarray(sidx16),
            "recip": np.ascontiguousarray(recip_p),
        })
    return in_maps, orders, k_sched


def kernel(features, neigh_idx, neigh_counts):
    from concourse.bass_utils import run_bass_kernel_spmd

    in_maps, orders, k_sched = prep_core_inputs(
        features, neigh_idx, neigh_counts)
    cw_scheds = _LAST_SCHED
    key = ("nc", N_NODES, BATCH // N_CORES, MAX_NEIGH, FEAT_DIM, k_sched)
    if key not in _KERNEL_CACHE:
        _KERNEL_CACHE[key] = build_nc(k_sched=list(k_sched),
                                      cw_scheds=cw_scheds)
    nc = _KERNEL_CACHE[key]

    res = run_bass_kernel_spmd(nc, in_maps, list(range(N_CORES)))
    b_loc = BATCH // N_CORES
    out = np.empty((BATCH, FEAT_DIM), dtype=np.float32)
    for c in range(N_CORES):
        out_c = np.empty((b_loc, FEAT_DIM), dtype=np.float32)
        out_c[orders[c]] = res.results[c]["out"]
        out[c * b_loc:(c + 1) * b_loc] = out_c
    return out
